# revision 24
# baseline (speedup 1.0000x reference)
"""GQA attention block (B=2, T=2048, C=2048, H=32, Hkv=8, D=64, RoPE, causal)
on 8 TRN2 NeuronCores.

Sharding: core = b*4 + g  (b = batch 0..1, g = head-group 0..3).
Each core computes 8 Q heads / 2 KV heads of one batch element:
  QKV projections -> RoPE -> causal softmax(QK^T/sqrt(D)) V -> partial
  output projection against its 512 columns of Wc.  Host sums the 4
  head-group bf16 partials per batch in f32.

Emission structure (per core): attention q-blocks are the spine; the
projections for t-block tb+2 and the output projection for rows of
block tb are chopped into small PE units and interleaved into attention
block tb+1's k-tile loop, so the PE always has long-stream matmul work
while the ScalarE exp (the attention-phase rate limiter) catches up.
Within a k-tile iteration: scores for kt+1 are emitted ahead (lookahead
1), the non-diagonal PV matmuls run one iteration late, the masked
diagonal PV two iterations late, and the y transposes four - each
deferral absorbs a cross-engine latency (ACT exp, Pool mask, DVE
normalize) without PE stalls.

Attention computes S^T = K Q^T tiles (k on partitions, head pair packed
at base partitions 0/64 = PE row groups); exp'd tiles (bf16) feed PV
matmuls with q on partitions: out[q, d] accumulates over k-tiles in two
PSUM banks (one start/stop per BANK - 'start' zeroes the whole bank's
has_written bits), with a ones-column in V accumulating the softmax
denominator per q row.  The normalize is then a per-partition
reciprocal+scale (DVE), PE-transposed into the y^T layout the output
projection consumes.  V is projected directly in [t, d] form (x chunk
as the stationary operand) - no separate transpose pass.

All DRAM inputs are pre-arranged host-side so each DMA moves >=2KB
contiguous per-partition lines; startup loads are need-ordered on the
SP queue.  RoPE staging is fp16 (DVE 2x) with the partition rotation
done by batched SBUF-SBUF DMAs (K first, so attention starts early).

Matmul operands are bf16 by default (KERNEL_MM_DTYPE=f32r selects
float32r: slower, lower error); PSUM accumulation is always fp32.
"""

import os

import ml_dtypes
import numpy as np

import concourse.bacc as bacc
import concourse.mybir as mybir
from concourse.tile import TileContext
from concourse.bass_utils import run_bass_kernel_spmd

B, T, C = 2, 2048, 2048
H, HKV, D = 32, 8, 64
ROPE_THETA = 10000.0

P = 128
NCT = C // P          # 16 contraction subtiles
TB = 512              # t-block width
NTB = T // TB         # 4
QB = 512              # q-block width in attention
KT = T // P           # 16 k-tiles
QH = H // 4           # 8 local q heads per core
LOCAL_HEADS = [0, 4, 1, 5, 2, 6, 3, 7]  # pair (p, p+4) shares a 128-row tile

F32 = mybir.dt.float32
F32R = mybir.dt.float32r
F16 = mybir.dt.float16
BF16 = mybir.dt.bfloat16

MM_MODE = os.environ.get("KERNEL_MM_DTYPE", "bf16")
MMDT = BF16 if MM_MODE == "bf16" else F32R
NPDT = ml_dtypes.bfloat16 if MM_MODE == "bf16" else np.float32

EXP_SCALE = float(1.0 / np.sqrt(D))


def build_bass():
    nc = bacc.Bacc("TRN2", target_bir_lowering=False, debug=False, num_devices=8)

    # All pre-arranged host-side: partition dim first, contiguous lines.
    xR = nc.dram_tensor("xR", [P, NTB, 4, 4 * TB], MMDT, kind="ExternalInput")
    wqR = nc.dram_tensor("wqR", [P, 4, NCT, P], MMDT, kind="ExternalInput")
    wkR = nc.dram_tensor("wkR", [P, NCT, 2 * D], MMDT, kind="ExternalInput")
    wvR = nc.dram_tensor("wvR", [P, NCT, 2 * D], MMDT, kind="ExternalInput")
    wcR = nc.dram_tensor("wcR", [P, 4, C], MMDT, kind="ExternalInput")
    cosT = nc.dram_tensor("cosT", [P, T], F16, kind="ExternalInput")
    sinT = nc.dram_tensor("sinT", [P, T], F16, kind="ExternalInput")
    tri = nc.dram_tensor("tri", [P, P], MMDT, kind="ExternalInput")
    ident = nc.dram_tensor("ident", [P, P], MMDT, kind="ExternalInput")
    vones = nc.dram_tensor("vones", [P, KT, 2], MMDT, kind="ExternalInput")
    # partials are stored bf16 (halves store traffic); the host sums the
    # four head-group partials per batch in f32
    out = nc.dram_tensor("out", [T, C], BF16, kind="ExternalOutput")

    with TileContext(nc) as tc:
        with (
            tc.tile_pool(name="persist", bufs=1) as persist,
            tc.tile_pool(name="xs", bufs=8) as xs,
            tc.tile_pool(name="rot", bufs=2) as rotp,
            tc.tile_pool(name="pt", bufs=5) as ptp,
            tc.tile_pool(name="yqd", bufs=8) as yqdp,
            tc.tile_pool(name="small", bufs=4) as small,
            tc.tile_pool(name="ostage", bufs=4) as ostage,
            tc.tile_pool(name="psMM", bufs=2, space="PSUM") as psMM,
            tc.tile_pool(name="psST", bufs=2, space="PSUM") as psST,
            tc.tile_pool(name="psPV", bufs=1, space="PSUM") as psPV,
        ):
            # ---- persistent SBUF tensors ------------------------------
            q_sb = persist.tile([P, 4, T], MMDT)          # Q^T (rope'd)
            k_sb = persist.tile([P, T], MMDT)             # K^T (rope'd)
            v_sb = persist.tile([P, KT, 2, D + 1], MMDT)  # V + ones col
            y_sb = persist.tile([P, 4, T], MMDT)          # attn out^T
            tri_sb = persist.tile([P, P], MMDT)
            id_sb = persist.tile([P, P], MMDT)
            cos_sb = persist.tile([P, T], F16)
            sin_sb = persist.tile([P, T], F16)
            wk_sb = persist.tile([P, NCT, 2 * D], MMDT, tag="wk")
            wv_sb = persist.tile([P, NCT, 2 * D], MMDT, tag="wv")
            wq_sb = persist.tile([P, 4, NCT, P], MMDT, tag="wq")
            wc_sb = persist.tile([P, 4, C], MMDT, tag="wc")

            # first-needed load: K-proj weights, in halves
            nc.sync.dma_start(wk_sb[:, 0:8], wkR[:, 0:8])

            # Warm the PE clock (HAM un-throttles after ~3.4us of activity)
            # with dummy matmuls while the first DMAs stream in, so the
            # first projection runs at full clock.
            wtile = persist.tile([P, 64], MMDT, tag="warm")
            nc.vector.memset(wtile[:], 0)
            wps = psMM.tile([P, 64], F32, tag="mm512", name="wps")
            for _ in range(60):
                nc.tensor.matmul(
                    wps[0:16, :], wtile[:, 0:16], wtile[:], start=True, stop=True
                )

            # work deferred from an attention block, flushed between the
            # following projection block's matmul groups so the PE stream
            # never waits on a fresh DVE result
            carry = []

            def flush_carry(n=1):
                for _ in range(min(n, len(carry))):
                    carry.pop(0)()

            def proj_units(tb):
                """Projection work for t-block tb as PE-unit emitters; the
                x/weight DMAs are issued immediately at creation."""
                tsl = slice(tb * TB, (tb + 1) * TB)
                # ---- x^T stream: 4 quarter-tiles of the contraction ---
                xh = []
                for qtr in range(4):
                    xb = xs.tile(
                        [P, NCT // 4, TB], MMDT, tag="xb", name=f"xb{qtr}"
                    )
                    if tb == 0 and qtr == 0:
                        # halves, so the first K matmuls start sooner
                        nc.sync.dma_start(xb[:, 0:2], xR[:, tb, qtr, 0 : 2 * TB])
                        nc.sync.dma_start(wk_sb[:, 8:16], wkR[:, 8:16])
                        nc.sync.dma_start(
                            xb[:, 2:4], xR[:, tb, qtr, 2 * TB : 4 * TB]
                        )
                        # V weights next: the V units run right after K
                        nc.sync.dma_start(wv_sb[:], wvR[:])
                    else:
                        nc.sync.dma_start(xb[:], xR[:, tb, qtr, :])
                    xh.append(xb)

                def xc(c):
                    return xh[c // (NCT // 4)][:, c % (NCT // 4), :]

                if tb == 0:
                    # small later-needed loads, on the ACT queue
                    nc.scalar.dma_start(tri_sb[:], tri[:])
                    nc.scalar.dma_start(id_sb[:], ident[:])
                    nc.scalar.dma_start(v_sb[:, :, :, D], vones[:])
                if tb == 1:
                    nc.sync.dma_start(wc_sb[:], wcR[:])

                # rope staging: 5 projection tiles (K, Q0..3) in fp16
                tmp5 = rotp.tile([P, 5, TB], F16, tag="rp_t")
                rtmp5 = rotp.tile([P, 5, TB], F16, tag="rp_r")

                # ---- K^T projection (one [128, TB] tile: 2 kv heads) --
                def rope(i, eng=None):
                    # K's rope splits its first mul onto the Pool engine so
                    # attention never waits on the DVE backlog for k_sb,
                    # without a long Pool chain delaying the causal masks
                    e = eng or nc.vector
                    dst = k_sb[:, tsl] if i == 0 else q_sb[:, i - 1, tsl]
                    e.tensor_mul(dst, tmp5[:, i, :], cos_sb[:, tsl])
                    nc.vector.tensor_mul(
                        rtmp5[:, i, :], rtmp5[:, i, :], sin_sb[:, tsl]
                    )
                    nc.vector.tensor_add(dst, dst, rtmp5[:, i, :])

                def rotate(lo, hi):
                    for olo, ilo in ((0, 32), (32, 0), (64, 96), (96, 64)):
                        nc.sync.dma_start(
                            rtmp5[olo : olo + 32, lo:hi, :],
                            tmp5[ilo : ilo + 32, lo:hi, :],
                        )

                def k_unit():
                    # K^T projection (one [128, TB] tile: 2 kv heads)
                    if tb == 0:
                        nc.sync.dma_start(cos_sb[:], cosT[:])
                        nc.sync.dma_start(sin_sb[:], sinT[:])
                    pk = psMM.tile([P, TB], F32, tag="mm512", name="pk")
                    for c in range(NCT):
                        nc.tensor.matmul(
                            pk[:], wk_sb[:, c, :], xc(c),
                            start=(c == 0), stop=(c == NCT - 1),
                        )
                    nc.vector.tensor_copy(tmp5[:, 0, :], pk[:])
                    # K rope first so attention can start before Q finishes
                    rotate(0, 1)
                    rope(0, eng=nc.gpsimd)

                def v_unit(s):
                    # V: direct [t, d] projection (lhsT = x chunk)
                    if tb == 0:
                        nc.sync.dma_start(wq_sb[:, s], wqR[:, s])
                    kt = tb * (TB // P) + s
                    pvd = psMM.tile([P, 2 * D], F32, tag="mm512", name="pvd")
                    for c in range(NCT):
                        nc.tensor.matmul(
                            pvd[:], xc(c)[:, s * P : (s + 1) * P],
                            wv_sb[:, c, :],
                            start=(c == 0), stop=(c == NCT - 1),
                        )
                    nc.vector.tensor_copy(
                        v_sb[:, kt, :, 0:D],
                        pvd[:].rearrange("p (h d) -> p h d", h=2),
                    )

                def q_unit(m):
                    pq = psMM.tile([P, TB], F32, tag="mm512", name="pq")
                    for c in range(NCT):
                        nc.tensor.matmul(
                            pq[:], wq_sb[:, m, c, :], xc(c),
                            start=(c == 0), stop=(c == NCT - 1),
                        )
                    nc.vector.tensor_copy(tmp5[:, 1 + m, :], pq[:])
                    # rope each Q tile as soon as its copy lands (one tile
                    # per unit, not batched in pairs): pr=0's scores - the
                    # attention spine's entry point - only wait for q_unit(0)
                    rotate(1 + m, 2 + m)
                    rope(1 + m)

                return (
                    [k_unit]
                    + [lambda s=s: v_unit(s) for s in range(4)]
                    + [lambda m=m: q_unit(m) for m in range(4)]
                )

            def outproj_units(jq):
                # ---- output projection for rows jq*TB..(jq+1)*TB, as 8
                # independent emitters (one [128, 1024] slab each) that the
                # next attention block interleaves into its PE stream ------
                obs = {}
                split = jq == NTB - 1

                def po_half(s, half, cbh):
                    # one [128, 512] column block: finer filler units keep
                    # the PE stream alternating between long matmuls and
                    # the weight-load-bound attention PV matmuls
                    t = jq * 4 + s
                    tsl = slice(t * P, (t + 1) * P)
                    split_store = split and s == 3
                    if cbh == 0:
                        obs[(s, half)] = ostage.tile(
                            [P, 2, 512], BF16, tag="ob", name="ob"
                        )
                    ob = obs[(s, half)]
                    cb = half * 2 + cbh
                    csl = slice(cb * 512, (cb + 1) * 512)
                    po = psMM.tile([P, 512], F32, tag="mm512", name="po")
                    for jj in range(4):
                        nc.tensor.matmul(
                            po[:],
                            y_sb[:, jj, tsl],
                            wc_sb[:, jj, csl],
                            start=(jj == 0),
                            stop=(jj == 3),
                        )
                        if jj < 3:
                            # generator midpoint: lets attention PV matmuls
                            # interleave between these long streams so the
                            # PE weight-load pull-ahead hides the PV
                            # LDWEIGHTS (HW effect; PSUM-safe because only
                            # psPV-bank matmuls run while this group is
                            # open, and the driver closes the generator
                            # before any other psMM allocation)
                            yield
                    nc.vector.tensor_copy(ob[:, cbh, :], po[:])
                    if split_store:
                        # tail: store each half-slab as soon as its copy
                        # lands, via the low-latency HWDGE queue
                        nc.sync.dma_start(out[tsl, csl], ob[:, cbh, :])
                    elif cbh == 1:
                        # all stores on the SP HWDGE queue: SWDGE stores
                        # would serialize with the causal masks on Pool
                        nc.sync.dma_start(
                            out[tsl, half * 1024 : (half + 1) * 1024], ob[:]
                        )

                return [
                    (lambda s=s, half=half, cbh=cbh: po_half(s, half, cbh))
                    for s in range(4)
                    for half in range(2)
                    for cbh in range(2)
                ]

            def attn_block(jq, fillers, tail_units=None):
                qb = jq * QB
                nkt = 4 * jq + 4
                niter = 4 * nkt
                nfill = len(fillers)
                it = 0
                delayed = []  # (emit_at_iteration, kind, pr, s, callable)
                gens = []  # mid-flight filler generators (split po units)

                def run_unit(u):
                    r = u()
                    if hasattr(r, "__next__"):
                        for _ in r:
                            pass

                def gen_step():
                    # advance one split filler by one 2-matmul burst
                    if gens:
                        try:
                            next(gens[0])
                        except StopIteration:
                            gens.pop(0)

                def gen_finish():
                    # close any open filler group before a psMM allocation
                    while gens:
                        try:
                            next(gens[0])
                        except StopIteration:
                            gens.pop(0)

                def flush_delayed(force=False):
                    while delayed and (force or delayed[0][0] <= it):
                        _, kind, dpr, ds, fn = delayed.pop(0)
                        fn()
                        if kind == "tr" and tail_units is not None and dpr == 3:
                            # final block: rows for subtile ds are complete
                            # across all head pairs -> emit its out-proj
                            for u in tail_units.pop(ds, []):
                                run_unit(u)

                for pr in range(4):  # head-pair tiles (local heads pr, pr+4)
                    # PV psum tiles are allocated AFTER the previous pr's
                    # drained readouts (see below) so the pool registers
                    # those reads as write-after-read predecessors
                    pvbox = {}

                    def pvt(s, pvbox=pvbox):
                        return pvbox["AB"[s >= 2]][:, s % 2]

                    def st_unit(kt, pr=pr):
                        # scores S^T for k-tile kt + exp (bf16 PSUM)
                        j = kt - 4 * jq
                        w = QB - P * j if j >= 0 else QB
                        qoff = qb + P * j if j >= 0 else qb
                        ksl = slice(kt * P, (kt + 1) * P)
                        st = psST.tile([P, 2, QB], F32, tag="st")
                        for hh in range(2):
                            hsl = slice(hh * D, (hh + 1) * D)
                            nc.tensor.matmul(
                                st[:, hh, 0:w],
                                k_sb[hsl, ksl],
                                q_sb[hsl, pr, qoff : qoff + w],
                                start=True,
                                stop=True,
                            )
                        ptile = ptp.tile([P, 2, QB], MMDT, tag="pt")
                        nc.scalar.activation(
                            ptile[:, :, 0:w],
                            st[:, :, 0:w],
                            mybir.ActivationFunctionType.Exp,
                            scale=EXP_SCALE,
                        )
                        if j >= 0:
                            nc.gpsimd.tensor_mul(
                                ptile[:, :, 0:P],
                                ptile[:, :, 0:P],
                                tri_sb[:, None, :].to_broadcast((P, 2, P)),
                            )
                        return ptile

                    def readout(s, pvt=pvt):
                        # 1/denominator, both heads in one op
                        rec = small.tile([P, 2], F32, tag="rec")
                        nc.vector.reciprocal_approx_fast(
                            rec[:], pvt(s)[:, :, D : D + 1]
                        )
                        yqd = yqdp.tile([P, 2 * D], MMDT, tag="yqd")
                        for hh in range(2):
                            nc.vector.tensor_scalar_mul(
                                yqd[:, hh * D : (hh + 1) * D],
                                pvt(s)[:, hh, 0:D],
                                rec[:, hh : hh + 1],
                            )
                        return yqd

                    def transpose_out(yqd, s, pr=pr):
                        # yqd was produced by readout(s); PE-transpose it
                        # into the y^T layout the output projection wants
                        ytr = psMM.tile([P, P], MMDT, tag="mm512", name="ytr")
                        nc.tensor.transpose(ytr[:], yqd[:], id_sb[:])
                        # exp is done by the last pr of the last block, so
                        # ACT is free there while DVE still has a backlog
                        eng = (
                            nc.scalar.copy
                            if (tail_units is not None and pr == 3)
                            else nc.vector.tensor_copy
                        )
                        eng(
                            y_sb[:, pr, qb + s * P : qb + (s + 1) * P], ytr[:]
                        )

                    def pv_mm(ptile, kt, s, j, pvt=pvt):
                        off = (s - max(j, 0)) * P
                        for hh in range(2):
                            # PSUM 'start' zeroes the whole bank's
                            # has_written bits, so exactly one start/stop
                            # per bank (pair of subtiles): intermediate
                            # writes overwrite-or-accumulate via the
                            # per-element has_written bits
                            nc.tensor.matmul(
                                pvt(s)[:, hh, 0 : D + 1],
                                ptile[:, hh, off : off + P],
                                v_sb[:, kt, hh, :],
                                start=(kt == 0 and s % 2 == 0 and hh == 0),
                                stop=(j == s and s % 2 == 1 and hh == 1),
                            )

                    def readout_pair(j, pr=pr, readout=readout, transpose_out=transpose_out):
                        # the bank holding subtiles (j-1, j) just closed
                        for s in (j - 1, j):
                            yqd = readout(s)
                            delayed.append(
                                (
                                    it + 4,
                                    "tr",
                                    pr,
                                    s,
                                    lambda y=yqd, s=s, f=transpose_out: f(y, s),
                                )
                            )

                    def diag_pv(p, kt, jj, pv_mm=pv_mm, readout_pair=readout_pair):
                        pv_mm(p, kt, jj, jj)
                        if jj in (1, 3):
                            readout_pair(jj)

                    def drain_pv(dpr):
                        # dpr's PSUM banks must close before this pr
                        # reopens them: run dpr's deferred diagonal-PV
                        snap = delayed[:]
                        del delayed[:]
                        kept = []
                        for e in snap:
                            if e[1] == "pv" and e[2] == dpr:
                                e[4]()  # may append fresh 'tr' entries
                            else:
                                kept.append(e)
                        delayed[:] = kept + delayed

                    ptile = st_unit(0)
                    if pr == 0:
                        # transposes carried from the previous block flush
                        # behind the first scores so the PE has work while
                        # their DVE inputs land
                        flush_carry(len(carry))
                    else:
                        # behind this pr's first scores: the Pool mask for
                        # the previous pr's last diagonal gets covered
                        drain_pv(pr - 1)
                    # PV psums, q on partitions: [q, subtile-pair, head,
                    # 128] padded to a full 2KB bank so matmul regions
                    # never straddle a bank boundary
                    pvbox["A"] = psPV.tile([P, 2, 2, P], F32, tag="pvA", name="pvA")
                    pvbox["B"] = psPV.tile([P, 2, 2, P], F32, tag="pvB", name="pvB")
                    prev = None  # (ptile, kt, j) whose PV runs next iter
                    for kt in range(nkt):
                        j = kt - 4 * jq  # >= 0: diagonal-crossing tile
                        nxt = st_unit(kt + 1) if kt + 1 < nkt else None
                        if j >= 0 and kt == 0:
                            # jq=0 only: the bank 'start' rides this diag
                            # matmul, so it must be emitted first
                            diag_pv(ptile, kt, j)
                        # non-diagonal PV runs one iteration late so exp
                        # always has a full S^T of slack; split filler
                        # bursts alternate with the PV pairs so the PE
                        # weight-load pull-ahead can hide PV LDWEIGHTS
                        if prev is not None:
                            pp, pkt, pj = prev
                            for s in range(
                                max(pj, 0) + (1 if pj >= 0 else 0), 4
                            ):
                                pv_mm(pp, pkt, s, pj)
                                gen_step()
                        if j >= 0 and kt > 0:
                            # the diagonal chunk additionally needs the
                            # Pool-engine mask; two iterations of slack
                            delayed.append(
                                (
                                    it + 3,
                                    "pv",
                                    pr,
                                    j,
                                    lambda p=ptile, kt=kt, jj=j, f=diag_pv: f(
                                        p, kt, jj
                                    ),
                                )
                            )
                        prev = (ptile, kt, j)
                        ptile = nxt
                        it += 1
                        gen_finish()  # close open groups before psMM allocs
                        flush_delayed()
                        # consume fillers proportionally across the block
                        while fillers and (nfill - len(fillers)) * niter < it * nfill:
                            g = fillers.pop(0)()
                            if hasattr(g, "__next__"):
                                try:
                                    next(g)  # first burst now, rest next iter
                                    gens.append(g)
                                except StopIteration:
                                    pass

                    gen_finish()
                    if prev is not None:
                        pp, pkt, pj = prev
                        for s in range(max(pj, 0) + (1 if pj >= 0 else 0), 4):
                            pv_mm(pp, pkt, s, pj)
                    if pr == 3:
                        drain_pv(pr)

                if tail_units is not None:
                    flush_delayed(force=True)
                else:
                    # run leftover diagonal-PV work now (it can append
                    # fresh readout/transpose entries); only transposes
                    # are carried into the next block's PE stream
                    while delayed:
                        _, kind, dpr, ds, fn = delayed.pop(0)
                        if kind == "pv":
                            fn()
                        else:
                            carry.append(fn)
                for fn in fillers:
                    run_unit(fn)
                del fillers[:]

            for u in proj_units(0):
                u()
            fillers = proj_units(1)
            for tb in range(NTB):
                tail = None
                if tb == NTB - 1:
                    units = outproj_units(tb)
                    tail = {s: units[4 * s : 4 * s + 4] for s in range(4)}
                attn_block(tb, fillers, tail)
                # Filler rebalance across the remaining attention blocks:
                # attn(1) gets proj(2)+out(0); attn(2) gets proj(3) only
                # (it is oversupplied anyway); attn(3) - whose exp overhang
                # is the largest - gets BOTH out(1) and out(2).
                a = proj_units(tb + 2) if tb + 2 < NTB else []
                if tb == 0:
                    b = outproj_units(0)
                elif tb == 1:
                    held = outproj_units(1)
                    b = held[:4]  # 6 units now; the rest feeds attn(3)
                elif tb == 2:
                    b = held[4:] + outproj_units(2)
                else:
                    b = []
                fillers = []
                while a or b:
                    if a:
                        fillers.append(a.pop(0))
                    if b:
                        fillers.append(b.pop(0))
            flush_carry(len(carry))

    nc.finalize()
    return nc


def _rope_tables(position_ids):
    t = position_ids.reshape(-1).astype(np.float64)  # [T]
    inv_freq = 1.0 / ROPE_THETA ** (np.arange(0, D, 2, dtype=np.float64) / D)
    freqs = np.outer(t, inv_freq)  # [T, D/2]
    cos = np.repeat(np.cos(freqs), 2, axis=1)  # [T, D] interleaved
    sin = np.repeat(np.sin(freqs), 2, axis=1)
    sign = np.where(np.arange(D) < D // 2, -1.0, 1.0)
    cosT = np.tile(cos.T, (2, 1)).astype(np.float16)            # [128, T]
    sinT = np.tile((sin * sign).T, (2, 1)).astype(np.float16)   # [128, T]
    return np.ascontiguousarray(cosT), np.ascontiguousarray(sinT)


def _head_perm(g):
    # row indices into Wq (and columns of Wc) for core head-group g
    rows = []
    for lh in LOCAL_HEADS:
        h = g * QH + lh
        rows.extend(range(h * D, (h + 1) * D))
    return np.asarray(rows)


def _part_major(wT, width):
    """[C, width] -> [128, NCT*width] with per-partition contiguous lines
    (line (c, m) = wT[c*128+p, m])."""
    return np.ascontiguousarray(
        wT.reshape(NCT, P, width).transpose(1, 0, 2).reshape(P, NCT * width)
    )


def make_in_maps(x, Wq, Wk, Wv, Wc, position_ids):
    x = np.asarray(x, dtype=np.float32)
    Wq = np.asarray(Wq, dtype=np.float32)
    Wk = np.asarray(Wk, dtype=np.float32)
    Wv = np.asarray(Wv, dtype=np.float32)
    Wc = np.asarray(Wc, dtype=np.float32)
    cosT, sinT = _rope_tables(np.asarray(position_ids))
    tri = np.triu(np.ones((P, P), dtype=np.float32))  # allow q >= k
    in_maps = []
    xr_cache = {}
    for core in range(8):
        b, g = divmod(core, 4)
        perm = _head_perm(g)
        kv = slice(2 * g * D, (2 * g + 2) * D)
        if b not in xr_cache:
            # xR[p, tb, qtr, o4*512+t] = x[b][tb*512+t, (qtr*4+o4)*128+p]
            xT = np.ascontiguousarray(x[b].T).astype(NPDT)  # [C, T]
            xr_cache[b] = np.ascontiguousarray(
                xT.reshape(4, 4, P, NTB, TB).transpose(2, 3, 0, 1, 4)
                .reshape(P, NTB, 4, 4 * TB)
            )
        wqT = np.ascontiguousarray(Wq[perm].T).astype(NPDT)  # [C, 512]
        in_maps.append(
            {
                "xR": xr_cache[b],
                # wqR[p, m, c, mm] = WqT[c*128+p, m*128+mm]
                "wqR": np.ascontiguousarray(
                    wqT.reshape(NCT, P, 4, P).transpose(1, 2, 0, 3)
                ),
                "wkR": _part_major(
                    np.ascontiguousarray(Wk[kv].T).astype(NPDT), 2 * D
                ),
                "wvR": _part_major(
                    np.ascontiguousarray(Wv[kv].T).astype(NPDT), 2 * D
                ),
                "wcR": np.ascontiguousarray(
                    Wc[:, perm].T.reshape(4, P, C).transpose(1, 0, 2)
                    .reshape(P, 4 * C)
                ).astype(NPDT),
                "cosT": cosT,
                "sinT": sinT,
                "tri": tri.astype(NPDT),
                "ident": np.eye(P, dtype=np.float32).astype(NPDT),
                "vones": np.ones((P, KT, 2), dtype=NPDT),
            }
        )
    return in_maps


_NC = None


def get_nc():
    global _NC
    if _NC is None:
        _NC = build_bass()
    return _NC


def run_cores(in_maps, core_ids, **kw):
    return run_bass_kernel_spmd(get_nc(), in_maps, core_ids=core_ids, **kw)


def kernel(x, Wq, Wk, Wv, Wc, position_ids, _trace=False, _res_out=None):
    in_maps = make_in_maps(x, Wq, Wk, Wv, Wc, position_ids)
    res = run_cores(in_maps, list(range(8)), trace=_trace)
    if _res_out is not None:
        _res_out.append(res)
    outs = [
        np.asarray(res.results[i]["out"], dtype=np.float32) for i in range(8)
    ]
    y = np.stack(
        [
            outs[0] + outs[1] + outs[2] + outs[3],
            outs[4] + outs[5] + outs[6] + outs[7],
        ]
    )
    return y.astype(np.float32)



# revision 25
# speedup vs baseline: 1.0417x; 1.0417x over previous
"""GQA attention block (B=2, T=2048, C=2048, H=32, Hkv=8, D=64, RoPE, causal)
on 8 TRN2 NeuronCores.

Sharding: core = b*4 + g  (b = batch 0..1, g = head-group 0..3).
Each core computes 8 Q heads / 2 KV heads of one batch element:
  QKV projections -> RoPE -> causal softmax(QK^T/sqrt(D)) V -> partial
  output projection against its 512 columns of Wc.  Host sums the 4
  head-group bf16 partials per batch in f32.

Emission structure (per core): attention q-blocks are the spine; the
projections for t-block tb+2 and the output projection for rows of
block tb are chopped into small PE units and interleaved into attention
block tb+1's k-tile loop, so the PE always has long-stream matmul work
while the ScalarE exp (the attention-phase rate limiter) catches up.
Within a k-tile iteration: scores for kt+1 are emitted ahead (lookahead
1), the non-diagonal PV matmuls run one iteration late, the masked
diagonal PV two iterations late, and the y transposes four - each
deferral absorbs a cross-engine latency (ACT exp, Pool mask, DVE
normalize) without PE stalls.

Attention computes S^T = K Q^T tiles (k on partitions, head pair packed
at base partitions 0/64 = PE row groups); exp'd tiles (bf16) feed PV
matmuls with q on partitions: out[q, d] accumulates over k-tiles in two
PSUM banks (one start/stop per BANK - 'start' zeroes the whole bank's
has_written bits), with a ones-column in V accumulating the softmax
denominator per q row.  The normalize is then a per-partition
reciprocal+scale (DVE), PE-transposed into the y^T layout the output
projection consumes.  V is projected directly in [t, d] form (x chunk
as the stationary operand) - no separate transpose pass.

All DRAM inputs are pre-arranged host-side so each DMA moves >=2KB
contiguous per-partition lines; startup loads are need-ordered on the
SP queue.  RoPE staging is fp16 (DVE 2x) with the partition rotation
done by batched SBUF-SBUF DMAs (K first, so attention starts early).

Matmul operands are bf16 by default (KERNEL_MM_DTYPE=f32r selects
float32r: slower, lower error); PSUM accumulation is always fp32.
"""

import os

import ml_dtypes
import numpy as np

import concourse.bacc as bacc
import concourse.mybir as mybir
from concourse.tile import TileContext
from concourse.bass_utils import run_bass_kernel_spmd

B, T, C = 2, 2048, 2048
H, HKV, D = 32, 8, 64
ROPE_THETA = 10000.0

P = 128
NCT = C // P          # 16 contraction subtiles
TB = 512              # t-block width
NTB = T // TB         # 4
QB = 512              # q-block width in attention
KT = T // P           # 16 k-tiles
QH = H // 4           # 8 local q heads per core
LOCAL_HEADS = [0, 4, 1, 5, 2, 6, 3, 7]  # pair (p, p+4) shares a 128-row tile

F32 = mybir.dt.float32
F32R = mybir.dt.float32r
F16 = mybir.dt.float16
BF16 = mybir.dt.bfloat16

MM_MODE = os.environ.get("KERNEL_MM_DTYPE", "bf16")
MMDT = BF16 if MM_MODE == "bf16" else F32R
NPDT = ml_dtypes.bfloat16 if MM_MODE == "bf16" else np.float32

EXP_SCALE = float(1.0 / np.sqrt(D))


def build_bass():
    nc = bacc.Bacc("TRN2", target_bir_lowering=False, debug=False, num_devices=8)

    # All pre-arranged host-side: partition dim first, contiguous lines.
    xR = nc.dram_tensor("xR", [P, NTB, 4, 4 * TB], MMDT, kind="ExternalInput")
    wqR = nc.dram_tensor("wqR", [P, 4, NCT, P], MMDT, kind="ExternalInput")
    wkR = nc.dram_tensor("wkR", [P, NCT, 2 * D], MMDT, kind="ExternalInput")
    wvR = nc.dram_tensor("wvR", [P, NCT, 2 * D], MMDT, kind="ExternalInput")
    wcR = nc.dram_tensor("wcR", [P, 4, C], MMDT, kind="ExternalInput")
    cosT = nc.dram_tensor("cosT", [P, T], F16, kind="ExternalInput")
    sinT = nc.dram_tensor("sinT", [P, T], F16, kind="ExternalInput")
    tri = nc.dram_tensor("tri", [P, P], MMDT, kind="ExternalInput")
    ident = nc.dram_tensor("ident", [P, P], MMDT, kind="ExternalInput")
    vones = nc.dram_tensor("vones", [P, KT, 2], MMDT, kind="ExternalInput")
    # partials are stored bf16 (halves store traffic); the host sums the
    # four head-group partials per batch in f32
    out = nc.dram_tensor("out", [T, C], BF16, kind="ExternalOutput")

    with TileContext(nc) as tc:
        with (
            tc.tile_pool(name="persist", bufs=1) as persist,
            tc.tile_pool(name="xs", bufs=8) as xs,
            tc.tile_pool(name="rot", bufs=2) as rotp,
            tc.tile_pool(name="pt", bufs=5) as ptp,
            tc.tile_pool(name="yqd", bufs=8) as yqdp,
            tc.tile_pool(name="small", bufs=4) as small,
            tc.tile_pool(name="ostage", bufs=4) as ostage,
            tc.tile_pool(name="psMM", bufs=2, space="PSUM") as psMM,
            tc.tile_pool(name="psST", bufs=2, space="PSUM") as psST,
            tc.tile_pool(name="psPV", bufs=1, space="PSUM") as psPV,
        ):
            # ---- persistent SBUF tensors ------------------------------
            q_sb = persist.tile([P, 4, T], MMDT)          # Q^T (rope'd)
            k_sb = persist.tile([P, T], MMDT)             # K^T (rope'd)
            v_sb = persist.tile([P, KT, 2, D + 1], MMDT)  # V + ones col
            y_sb = persist.tile([P, 4, T], MMDT)          # attn out^T
            tri_sb = persist.tile([P, P], MMDT)
            id_sb = persist.tile([P, P], MMDT)
            cos_sb = persist.tile([P, T], F16)
            sin_sb = persist.tile([P, T], F16)
            wk_sb = persist.tile([P, NCT, 2 * D], MMDT, tag="wk")
            wv_sb = persist.tile([P, NCT, 2 * D], MMDT, tag="wv")
            wq_sb = persist.tile([P, 4, NCT, P], MMDT, tag="wq")
            wc_sb = persist.tile([P, 4, C], MMDT, tag="wc")

            # first-needed load: K-proj weights, in halves
            nc.sync.dma_start(wk_sb[:, 0:8], wkR[:, 0:8])

            # Warm the PE clock (HAM un-throttles after ~3.4us of activity)
            # with dummy matmuls while the first DMAs stream in, so the
            # first projection runs at full clock.
            wtile = persist.tile([P, 64], MMDT, tag="warm")
            nc.vector.memset(wtile[:], 0)
            wps = psMM.tile([P, 64], F32, tag="mm512", name="wps")
            for _ in range(60):
                nc.tensor.matmul(
                    wps[0:16, :], wtile[:, 0:16], wtile[:], start=True, stop=True
                )

            # work deferred from an attention block, flushed between the
            # following projection block's matmul groups so the PE stream
            # never waits on a fresh DVE result
            carry = []

            def flush_carry(n=1):
                for _ in range(min(n, len(carry))):
                    carry.pop(0)()

            def proj_units(tb):
                """Projection work for t-block tb as PE-unit emitters; the
                x/weight DMAs are issued immediately at creation."""
                tsl = slice(tb * TB, (tb + 1) * TB)
                # ---- x^T stream: 4 quarter-tiles of the contraction ---
                xh = []
                for qtr in range(4):
                    xb = xs.tile(
                        [P, NCT // 4, TB], MMDT, tag="xb", name=f"xb{qtr}"
                    )
                    if tb == 0 and qtr == 0:
                        # halves, so the first K matmuls start sooner
                        nc.sync.dma_start(xb[:, 0:2], xR[:, tb, qtr, 0 : 2 * TB])
                        nc.sync.dma_start(wk_sb[:, 8:16], wkR[:, 8:16])
                        nc.sync.dma_start(
                            xb[:, 2:4], xR[:, tb, qtr, 2 * TB : 4 * TB]
                        )
                        # V weights next: the V units run right after K
                        nc.sync.dma_start(wv_sb[:], wvR[:])
                    else:
                        nc.sync.dma_start(xb[:], xR[:, tb, qtr, :])
                    xh.append(xb)

                def xc(c):
                    return xh[c // (NCT // 4)][:, c % (NCT // 4), :]

                if tb == 0:
                    # small later-needed loads, on the ACT queue
                    nc.scalar.dma_start(tri_sb[:], tri[:])
                    nc.scalar.dma_start(id_sb[:], ident[:])
                    nc.scalar.dma_start(v_sb[:, :, :, D], vones[:])
                if tb == 1:
                    nc.sync.dma_start(wc_sb[:], wcR[:])

                # rope staging: 5 projection tiles (K, Q0..3) in fp16
                tmp5 = rotp.tile([P, 5, TB], F16, tag="rp_t")
                rtmp5 = rotp.tile([P, 5, TB], F16, tag="rp_r")

                # ---- K^T projection (one [128, TB] tile: 2 kv heads) --
                def rope(i, eng=None):
                    # K's rope splits its first mul onto the Pool engine so
                    # attention never waits on the DVE backlog for k_sb,
                    # without a long Pool chain delaying the causal masks
                    e = eng or nc.vector
                    dst = k_sb[:, tsl] if i == 0 else q_sb[:, i - 1, tsl]
                    e.tensor_mul(dst, tmp5[:, i, :], cos_sb[:, tsl])
                    nc.vector.tensor_mul(
                        rtmp5[:, i, :], rtmp5[:, i, :], sin_sb[:, tsl]
                    )
                    nc.vector.tensor_add(dst, dst, rtmp5[:, i, :])

                def rotate(lo, hi):
                    for olo, ilo in ((0, 32), (32, 0), (64, 96), (96, 64)):
                        nc.sync.dma_start(
                            rtmp5[olo : olo + 32, lo:hi, :],
                            tmp5[ilo : ilo + 32, lo:hi, :],
                        )

                def k_unit():
                    # K^T projection (one [128, TB] tile: 2 kv heads)
                    if tb == 0:
                        nc.sync.dma_start(cos_sb[:], cosT[:])
                        nc.sync.dma_start(sin_sb[:], sinT[:])
                    pk = psMM.tile([P, TB], F32, tag="mm512", name="pk")
                    for c in range(NCT):
                        nc.tensor.matmul(
                            pk[:], wk_sb[:, c, :], xc(c),
                            start=(c == 0), stop=(c == NCT - 1),
                        )
                    nc.vector.tensor_copy(tmp5[:, 0, :], pk[:])
                    # K rope first so attention can start before Q finishes
                    rotate(0, 1)
                    rope(0, eng=nc.gpsimd)

                def v_unit(s):
                    # V: direct [t, d] projection (lhsT = x chunk)
                    if tb == 0:
                        nc.sync.dma_start(wq_sb[:, s], wqR[:, s])
                    kt = tb * (TB // P) + s
                    pvd = psMM.tile([P, 2 * D], F32, tag="mm512", name="pvd")
                    for c in range(NCT):
                        nc.tensor.matmul(
                            pvd[:], xc(c)[:, s * P : (s + 1) * P],
                            wv_sb[:, c, :],
                            start=(c == 0), stop=(c == NCT - 1),
                        )
                    nc.vector.tensor_copy(
                        v_sb[:, kt, :, 0:D],
                        pvd[:].rearrange("p (h d) -> p h d", h=2),
                    )

                def q_unit(m):
                    pq = psMM.tile([P, TB], F32, tag="mm512", name="pq")
                    for c in range(NCT):
                        nc.tensor.matmul(
                            pq[:], wq_sb[:, m, c, :], xc(c),
                            start=(c == 0), stop=(c == NCT - 1),
                        )
                    nc.vector.tensor_copy(tmp5[:, 1 + m, :], pq[:])
                    if m == 1:
                        rotate(1, 3)
                        rope(1)
                        rope(2)
                    if m == 3:
                        rotate(3, 5)
                        rope(3)
                        rope(4)

                return (
                    [k_unit]
                    + [lambda s=s: v_unit(s) for s in range(4)]
                    + [lambda m=m: q_unit(m) for m in range(4)]
                )

            def outproj_units(jq):
                # ---- output projection for rows jq*TB..(jq+1)*TB, as 8
                # independent emitters (one [128, 1024] slab each) that the
                # next attention block interleaves into its PE stream ------
                obs = {}
                split = jq == NTB - 1

                def po_half(s, half, cbh):
                    # one [128, 512] column block: finer filler units keep
                    # the PE stream alternating between long matmuls and
                    # the weight-load-bound attention PV matmuls
                    t = jq * 4 + s
                    tsl = slice(t * P, (t + 1) * P)
                    split_store = split and s == 3
                    if cbh == 0:
                        obs[(s, half)] = ostage.tile(
                            [P, 2, 512], BF16, tag="ob", name="ob"
                        )
                    ob = obs[(s, half)]
                    cb = half * 2 + cbh
                    csl = slice(cb * 512, (cb + 1) * 512)
                    po = psMM.tile([P, 512], F32, tag="mm512", name="po")
                    for jj in range(4):
                        nc.tensor.matmul(
                            po[:],
                            y_sb[:, jj, tsl],
                            wc_sb[:, jj, csl],
                            start=(jj == 0),
                            stop=(jj == 3),
                        )
                        if jj < 3:
                            # generator midpoint: lets attention PV matmuls
                            # interleave between these long streams so the
                            # PE weight-load pull-ahead hides the PV
                            # LDWEIGHTS (HW effect; PSUM-safe because only
                            # psPV-bank matmuls run while this group is
                            # open, and the driver closes the generator
                            # before any other psMM allocation)
                            yield
                    nc.vector.tensor_copy(ob[:, cbh, :], po[:])
                    if split_store:
                        # tail: store each half-slab as soon as its copy
                        # lands, via the low-latency HWDGE queue
                        nc.sync.dma_start(out[tsl, csl], ob[:, cbh, :])
                    elif cbh == 1:
                        # all stores on the SP HWDGE queue: SWDGE stores
                        # would serialize with the causal masks on Pool
                        nc.sync.dma_start(
                            out[tsl, half * 1024 : (half + 1) * 1024], ob[:]
                        )

                return [
                    (lambda s=s, half=half, cbh=cbh: po_half(s, half, cbh))
                    for s in range(4)
                    for half in range(2)
                    for cbh in range(2)
                ]

            def attn_block(jq, fillers, tail_units=None):
                qb = jq * QB
                nkt = 4 * jq + 4
                niter = 4 * nkt
                nfill = len(fillers)
                it = 0
                delayed = []  # (emit_at_iteration, kind, pr, s, callable)
                gens = []  # mid-flight filler generators (split po units)

                def run_unit(u):
                    r = u()
                    if hasattr(r, "__next__"):
                        for _ in r:
                            pass

                def gen_step():
                    # advance one split filler by one 2-matmul burst
                    if gens:
                        try:
                            next(gens[0])
                        except StopIteration:
                            gens.pop(0)

                def gen_finish():
                    # close any open filler group before a psMM allocation
                    while gens:
                        try:
                            next(gens[0])
                        except StopIteration:
                            gens.pop(0)

                def flush_delayed(force=False):
                    while delayed and (force or delayed[0][0] <= it):
                        _, kind, dpr, ds, fn = delayed.pop(0)
                        fn()
                        if kind == "tr" and tail_units is not None and dpr == 3:
                            # final block: rows for subtile ds are complete
                            # across all head pairs -> emit its out-proj
                            for u in tail_units.pop(ds, []):
                                run_unit(u)

                for pr in range(4):  # head-pair tiles (local heads pr, pr+4)
                    # PV psum tiles are allocated AFTER the previous pr's
                    # drained readouts (see below) so the pool registers
                    # those reads as write-after-read predecessors
                    pvbox = {}

                    def pvt(s, pvbox=pvbox):
                        return pvbox["AB"[s >= 2]][:, s % 2]

                    def st_unit(kt, pr=pr):
                        # scores S^T for k-tile kt + exp (bf16 PSUM)
                        j = kt - 4 * jq
                        w = QB - P * j if j >= 0 else QB
                        qoff = qb + P * j if j >= 0 else qb
                        ksl = slice(kt * P, (kt + 1) * P)
                        st = psST.tile([P, 2, QB], F32, tag="st")
                        for hh in range(2):
                            hsl = slice(hh * D, (hh + 1) * D)
                            nc.tensor.matmul(
                                st[:, hh, 0:w],
                                k_sb[hsl, ksl],
                                q_sb[hsl, pr, qoff : qoff + w],
                                start=True,
                                stop=True,
                            )
                        ptile = ptp.tile([P, 2, QB], MMDT, tag="pt")
                        nc.scalar.activation(
                            ptile[:, :, 0:w],
                            st[:, :, 0:w],
                            mybir.ActivationFunctionType.Exp,
                            scale=EXP_SCALE,
                        )
                        if j >= 0:
                            nc.gpsimd.tensor_mul(
                                ptile[:, :, 0:P],
                                ptile[:, :, 0:P],
                                tri_sb[:, None, :].to_broadcast((P, 2, P)),
                            )
                        return ptile

                    def readout(s, pvt=pvt):
                        # 1/denominator, both heads in one op
                        rec = small.tile([P, 2], F32, tag="rec")
                        nc.vector.reciprocal_approx_fast(
                            rec[:], pvt(s)[:, :, D : D + 1]
                        )
                        yqd = yqdp.tile([P, 2 * D], MMDT, tag="yqd")
                        for hh in range(2):
                            nc.vector.tensor_scalar_mul(
                                yqd[:, hh * D : (hh + 1) * D],
                                pvt(s)[:, hh, 0:D],
                                rec[:, hh : hh + 1],
                            )
                        return yqd

                    def transpose_out(yqd, s, pr=pr):
                        # yqd was produced by readout(s); PE-transpose it
                        # into the y^T layout the output projection wants
                        ytr = psMM.tile([P, P], MMDT, tag="mm512", name="ytr")
                        nc.tensor.transpose(ytr[:], yqd[:], id_sb[:])
                        # exp is done by the last pr of the last block, so
                        # ACT is free there while DVE still has a backlog
                        eng = (
                            nc.scalar.copy
                            if (tail_units is not None and pr == 3)
                            else nc.vector.tensor_copy
                        )
                        eng(
                            y_sb[:, pr, qb + s * P : qb + (s + 1) * P], ytr[:]
                        )

                    def pv_mm(ptile, kt, s, j, pvt=pvt):
                        off = (s - max(j, 0)) * P
                        for hh in range(2):
                            # PSUM 'start' zeroes the whole bank's
                            # has_written bits, so exactly one start/stop
                            # per bank (pair of subtiles): intermediate
                            # writes overwrite-or-accumulate via the
                            # per-element has_written bits
                            nc.tensor.matmul(
                                pvt(s)[:, hh, 0 : D + 1],
                                ptile[:, hh, off : off + P],
                                v_sb[:, kt, hh, :],
                                start=(kt == 0 and s % 2 == 0 and hh == 0),
                                stop=(j == s and s % 2 == 1 and hh == 1),
                            )

                    def readout_pair(j, pr=pr, readout=readout, transpose_out=transpose_out):
                        # the bank holding subtiles (j-1, j) just closed
                        for s in (j - 1, j):
                            yqd = readout(s)
                            delayed.append(
                                (
                                    it + 4,
                                    "tr",
                                    pr,
                                    s,
                                    lambda y=yqd, s=s, f=transpose_out: f(y, s),
                                )
                            )

                    def diag_pv(p, kt, jj, pv_mm=pv_mm, readout_pair=readout_pair):
                        pv_mm(p, kt, jj, jj)
                        if jj in (1, 3):
                            readout_pair(jj)

                    def drain_pv(dpr):
                        # dpr's PSUM banks must close before this pr
                        # reopens them: run dpr's deferred diagonal-PV
                        snap = delayed[:]
                        del delayed[:]
                        kept = []
                        for e in snap:
                            if e[1] == "pv" and e[2] == dpr:
                                e[4]()  # may append fresh 'tr' entries
                            else:
                                kept.append(e)
                        delayed[:] = kept + delayed

                    ptile = st_unit(0)
                    if pr == 0:
                        # transposes carried from the previous block flush
                        # behind the first scores so the PE has work while
                        # their DVE inputs land
                        flush_carry(len(carry))
                    else:
                        # behind this pr's first scores: the Pool mask for
                        # the previous pr's last diagonal gets covered
                        drain_pv(pr - 1)
                    # PV psums, q on partitions: [q, subtile-pair, head,
                    # 128] padded to a full 2KB bank so matmul regions
                    # never straddle a bank boundary
                    pvbox["A"] = psPV.tile([P, 2, 2, P], F32, tag="pvA", name="pvA")
                    pvbox["B"] = psPV.tile([P, 2, 2, P], F32, tag="pvB", name="pvB")
                    prev = None  # (ptile, kt, j) whose PV runs next iter
                    for kt in range(nkt):
                        j = kt - 4 * jq  # >= 0: diagonal-crossing tile
                        nxt = st_unit(kt + 1) if kt + 1 < nkt else None
                        if j >= 0 and kt == 0:
                            # jq=0 only: the bank 'start' rides this diag
                            # matmul, so it must be emitted first
                            diag_pv(ptile, kt, j)
                        # non-diagonal PV runs one iteration late so exp
                        # always has a full S^T of slack; split filler
                        # bursts alternate with the PV pairs so the PE
                        # weight-load pull-ahead can hide PV LDWEIGHTS
                        if prev is not None:
                            pp, pkt, pj = prev
                            for s in range(
                                max(pj, 0) + (1 if pj >= 0 else 0), 4
                            ):
                                pv_mm(pp, pkt, s, pj)
                                gen_step()
                        if j >= 0 and kt > 0:
                            # the diagonal chunk additionally needs the
                            # Pool-engine mask; two iterations of slack
                            delayed.append(
                                (
                                    it + 3,
                                    "pv",
                                    pr,
                                    j,
                                    lambda p=ptile, kt=kt, jj=j, f=diag_pv: f(
                                        p, kt, jj
                                    ),
                                )
                            )
                        prev = (ptile, kt, j)
                        ptile = nxt
                        it += 1
                        gen_finish()  # close open groups before psMM allocs
                        flush_delayed()
                        # consume fillers proportionally across the block
                        while fillers and (nfill - len(fillers)) * niter < it * nfill:
                            g = fillers.pop(0)()
                            if hasattr(g, "__next__"):
                                try:
                                    next(g)  # first burst now, rest next iter
                                    gens.append(g)
                                except StopIteration:
                                    pass

                    gen_finish()
                    if prev is not None:
                        pp, pkt, pj = prev
                        for s in range(max(pj, 0) + (1 if pj >= 0 else 0), 4):
                            pv_mm(pp, pkt, s, pj)
                    if pr == 3:
                        drain_pv(pr)

                if tail_units is not None:
                    flush_delayed(force=True)
                else:
                    # run leftover diagonal-PV work now (it can append
                    # fresh readout/transpose entries); only transposes
                    # are carried into the next block's PE stream
                    while delayed:
                        _, kind, dpr, ds, fn = delayed.pop(0)
                        if kind == "pv":
                            fn()
                        else:
                            carry.append(fn)
                for fn in fillers:
                    run_unit(fn)
                del fillers[:]

            for u in proj_units(0):
                u()
            fillers = proj_units(1)
            for tb in range(NTB):
                tail = None
                if tb == NTB - 1:
                    units = outproj_units(tb)
                    tail = {s: units[4 * s : 4 * s + 4] for s in range(4)}
                attn_block(tb, fillers, tail)
                # Filler rebalance across the remaining attention blocks:
                # attn(1) gets proj(2)+out(0); attn(2) gets proj(3) only
                # (it is oversupplied anyway); attn(3) - whose exp overhang
                # is the largest - gets BOTH out(1) and out(2).
                a = proj_units(tb + 2) if tb + 2 < NTB else []
                if tb == 0:
                    b = outproj_units(0)
                elif tb == 1:
                    held = outproj_units(1)
                    b = held[:4]  # 6 units now; the rest feeds attn(3)
                elif tb == 2:
                    b = held[4:] + outproj_units(2)
                else:
                    b = []
                fillers = []
                while a or b:
                    if a:
                        fillers.append(a.pop(0))
                    if b:
                        fillers.append(b.pop(0))
            flush_carry(len(carry))

    nc.finalize()
    return nc


def _rope_tables(position_ids):
    t = position_ids.reshape(-1).astype(np.float64)  # [T]
    inv_freq = 1.0 / ROPE_THETA ** (np.arange(0, D, 2, dtype=np.float64) / D)
    freqs = np.outer(t, inv_freq)  # [T, D/2]
    cos = np.repeat(np.cos(freqs), 2, axis=1)  # [T, D] interleaved
    sin = np.repeat(np.sin(freqs), 2, axis=1)
    sign = np.where(np.arange(D) < D // 2, -1.0, 1.0)
    cosT = np.tile(cos.T, (2, 1)).astype(np.float16)            # [128, T]
    sinT = np.tile((sin * sign).T, (2, 1)).astype(np.float16)   # [128, T]
    return np.ascontiguousarray(cosT), np.ascontiguousarray(sinT)


def _head_perm(g):
    # row indices into Wq (and columns of Wc) for core head-group g
    rows = []
    for lh in LOCAL_HEADS:
        h = g * QH + lh
        rows.extend(range(h * D, (h + 1) * D))
    return np.asarray(rows)


def _part_major(wT, width):
    """[C, width] -> [128, NCT*width] with per-partition contiguous lines
    (line (c, m) = wT[c*128+p, m])."""
    return np.ascontiguousarray(
        wT.reshape(NCT, P, width).transpose(1, 0, 2).reshape(P, NCT * width)
    )


def make_in_maps(x, Wq, Wk, Wv, Wc, position_ids):
    x = np.asarray(x, dtype=np.float32)
    Wq = np.asarray(Wq, dtype=np.float32)
    Wk = np.asarray(Wk, dtype=np.float32)
    Wv = np.asarray(Wv, dtype=np.float32)
    Wc = np.asarray(Wc, dtype=np.float32)
    cosT, sinT = _rope_tables(np.asarray(position_ids))
    tri = np.triu(np.ones((P, P), dtype=np.float32))  # allow q >= k
    in_maps = []
    xr_cache = {}
    for core in range(8):
        b, g = divmod(core, 4)
        perm = _head_perm(g)
        kv = slice(2 * g * D, (2 * g + 2) * D)
        if b not in xr_cache:
            # xR[p, tb, qtr, o4*512+t] = x[b][tb*512+t, (qtr*4+o4)*128+p]
            xT = np.ascontiguousarray(x[b].T).astype(NPDT)  # [C, T]
            xr_cache[b] = np.ascontiguousarray(
                xT.reshape(4, 4, P, NTB, TB).transpose(2, 3, 0, 1, 4)
                .reshape(P, NTB, 4, 4 * TB)
            )
        wqT = np.ascontiguousarray(Wq[perm].T).astype(NPDT)  # [C, 512]
        in_maps.append(
            {
                "xR": xr_cache[b],
                # wqR[p, m, c, mm] = WqT[c*128+p, m*128+mm]
                "wqR": np.ascontiguousarray(
                    wqT.reshape(NCT, P, 4, P).transpose(1, 2, 0, 3)
                ),
                "wkR": _part_major(
                    np.ascontiguousarray(Wk[kv].T).astype(NPDT), 2 * D
                ),
                "wvR": _part_major(
                    np.ascontiguousarray(Wv[kv].T).astype(NPDT), 2 * D
                ),
                "wcR": np.ascontiguousarray(
                    Wc[:, perm].T.reshape(4, P, C).transpose(1, 0, 2)
                    .reshape(P, 4 * C)
                ).astype(NPDT),
                "cosT": cosT,
                "sinT": sinT,
                "tri": tri.astype(NPDT),
                "ident": np.eye(P, dtype=np.float32).astype(NPDT),
                "vones": np.ones((P, KT, 2), dtype=NPDT),
            }
        )
    return in_maps


_NC = None


def get_nc():
    global _NC
    if _NC is None:
        _NC = build_bass()
    return _NC


def run_cores(in_maps, core_ids, **kw):
    return run_bass_kernel_spmd(get_nc(), in_maps, core_ids=core_ids, **kw)


def kernel(x, Wq, Wk, Wv, Wc, position_ids, _trace=False, _res_out=None):
    in_maps = make_in_maps(x, Wq, Wk, Wv, Wc, position_ids)
    res = run_cores(in_maps, list(range(8)), trace=_trace)
    if _res_out is not None:
        _res_out.append(res)
    outs = [
        np.asarray(res.results[i]["out"], dtype=np.float32) for i in range(8)
    ]
    y = np.stack(
        [
            outs[0] + outs[1] + outs[2] + outs[3],
            outs[4] + outs[5] + outs[6] + outs[7],
        ]
    )
    return y.astype(np.float32)



# revision 27
# speedup vs baseline: 1.0453x; 1.0035x over previous
"""GQA attention block (B=2, T=2048, C=2048, H=32, Hkv=8, D=64, RoPE, causal)
on 8 TRN2 NeuronCores.

Sharding: core = b*4 + g  (b = batch 0..1, g = head-group 0..3).
Each core computes 8 Q heads / 2 KV heads of one batch element:
  QKV projections -> RoPE -> causal softmax(QK^T/sqrt(D)) V -> partial
  output projection against its 512 columns of Wc.  Host sums the 4
  head-group bf16 partials per batch in f32.

Emission structure (per core): attention q-blocks are the spine; the
projections for t-block tb+2 and the output projection for rows of
block tb are chopped into small PE units and interleaved into attention
block tb+1's k-tile loop, so the PE always has long-stream matmul work
while the ScalarE exp (the attention-phase rate limiter) catches up.
Within a k-tile iteration: scores for kt+1 are emitted ahead (lookahead
1), the non-diagonal PV matmuls run one iteration late, the masked
diagonal PV two iterations late, and the y transposes four - each
deferral absorbs a cross-engine latency (ACT exp, Pool mask, DVE
normalize) without PE stalls.

Attention computes S^T = K Q^T tiles (k on partitions, head pair packed
at base partitions 0/64 = PE row groups); exp'd tiles (bf16) feed PV
matmuls with q on partitions: out[q, d] accumulates over k-tiles in two
PSUM banks (one start/stop per BANK - 'start' zeroes the whole bank's
has_written bits), with a ones-column in V accumulating the softmax
denominator per q row.  The normalize is then a per-partition
reciprocal+scale (DVE), PE-transposed into the y^T layout the output
projection consumes.  V is projected directly in [t, d] form (x chunk
as the stationary operand) - no separate transpose pass.

All DRAM inputs are pre-arranged host-side so each DMA moves >=2KB
contiguous per-partition lines; startup loads are need-ordered on the
SP queue.  RoPE staging is fp16 (DVE 2x) with the partition rotation
done by batched SBUF-SBUF DMAs (K first, so attention starts early).

Matmul operands are bf16 by default (KERNEL_MM_DTYPE=f32r selects
float32r: slower, lower error); PSUM accumulation is always fp32.
"""

import os

import ml_dtypes
import numpy as np

import concourse.bacc as bacc
import concourse.mybir as mybir
from concourse.tile import TileContext
from concourse.bass_utils import run_bass_kernel_spmd

B, T, C = 2, 2048, 2048
H, HKV, D = 32, 8, 64
ROPE_THETA = 10000.0

P = 128
NCT = C // P          # 16 contraction subtiles
TB = 512              # t-block width
NTB = T // TB         # 4
QB = 512              # q-block width in attention
KT = T // P           # 16 k-tiles
QH = H // 4           # 8 local q heads per core
LOCAL_HEADS = [0, 4, 1, 5, 2, 6, 3, 7]  # pair (p, p+4) shares a 128-row tile

F32 = mybir.dt.float32
F32R = mybir.dt.float32r
F16 = mybir.dt.float16
BF16 = mybir.dt.bfloat16

MM_MODE = os.environ.get("KERNEL_MM_DTYPE", "bf16")
MMDT = BF16 if MM_MODE == "bf16" else F32R
NPDT = ml_dtypes.bfloat16 if MM_MODE == "bf16" else np.float32

EXP_SCALE = float(1.0 / np.sqrt(D))


def build_bass():
    nc = bacc.Bacc("TRN2", target_bir_lowering=False, debug=False, num_devices=8)

    # All pre-arranged host-side: partition dim first, contiguous lines.
    xR = nc.dram_tensor("xR", [P, NTB, 4, 4 * TB], MMDT, kind="ExternalInput")
    wqR = nc.dram_tensor("wqR", [P, 4, NCT, P], MMDT, kind="ExternalInput")
    wkR = nc.dram_tensor("wkR", [P, NCT, 2 * D], MMDT, kind="ExternalInput")
    wvR = nc.dram_tensor("wvR", [P, NCT, 2 * D], MMDT, kind="ExternalInput")
    wcR = nc.dram_tensor("wcR", [P, 4, C], MMDT, kind="ExternalInput")
    cosT = nc.dram_tensor("cosT", [P, T], F16, kind="ExternalInput")
    sinT = nc.dram_tensor("sinT", [P, T], F16, kind="ExternalInput")
    tri = nc.dram_tensor("tri", [P, P], MMDT, kind="ExternalInput")
    ident = nc.dram_tensor("ident", [P, P], MMDT, kind="ExternalInput")
    vones = nc.dram_tensor("vones", [P, KT, 2], MMDT, kind="ExternalInput")
    # partials are stored bf16 (halves store traffic); the host sums the
    # four head-group partials per batch in f32
    out = nc.dram_tensor("out", [T, C], BF16, kind="ExternalOutput")

    with TileContext(nc) as tc:
        with (
            tc.tile_pool(name="persist", bufs=1) as persist,
            tc.tile_pool(name="xs", bufs=8) as xs,
            tc.tile_pool(name="rot", bufs=2) as rotp,
            tc.tile_pool(name="pt", bufs=5) as ptp,
            tc.tile_pool(name="yqd", bufs=8) as yqdp,
            tc.tile_pool(name="small", bufs=4) as small,
            tc.tile_pool(name="ostage", bufs=4) as ostage,
            tc.tile_pool(name="psMM", bufs=2, space="PSUM") as psMM,
            tc.tile_pool(name="psST", bufs=2, space="PSUM") as psST,
            tc.tile_pool(name="psPV", bufs=1, space="PSUM") as psPV,
        ):
            # ---- persistent SBUF tensors ------------------------------
            q_sb = persist.tile([P, 4, T], MMDT)          # Q^T (rope'd)
            k_sb = persist.tile([P, T], MMDT)             # K^T (rope'd)
            v_sb = persist.tile([P, KT, 2, D + 1], MMDT)  # V + ones col
            y_sb = persist.tile([P, 4, T], MMDT)          # attn out^T
            tri_sb = persist.tile([P, P], MMDT)
            id_sb = persist.tile([P, P], MMDT)
            cos_sb = persist.tile([P, T], F16)
            sin_sb = persist.tile([P, T], F16)
            wk_sb = persist.tile([P, NCT, 2 * D], MMDT, tag="wk")
            wv_sb = persist.tile([P, NCT, 2 * D], MMDT, tag="wv")
            wq_sb = persist.tile([P, 4, NCT, P], MMDT, tag="wq")
            wc_sb = persist.tile([P, 4, C], MMDT, tag="wc")

            # first-needed load: K-proj weights.  The leading chunk is
            # small (64 KB): the DMA path drains at only ~85 B/ns while
            # ramping up, and the first K matmul needs just wk[0:2] plus
            # one x chunk - fine granularity here starts the PE ~1.5us
            # earlier (dependency tracking is region-exact).
            nc.sync.dma_start(wk_sb[:, 0:2], wkR[:, 0:2])

            # Warm the PE clock (HAM un-throttles after ~3.4us of activity)
            # with dummy matmuls while the first DMAs stream in, so the
            # first projection runs at full clock.
            wtile = persist.tile([P, 64], MMDT, tag="warm")
            nc.vector.memset(wtile[:], 0)
            wps = psMM.tile([P, 64], F32, tag="mm512", name="wps")
            for _ in range(60):
                nc.tensor.matmul(
                    wps[0:16, :], wtile[:, 0:16], wtile[:], start=True, stop=True
                )

            # work deferred from an attention block, flushed between the
            # following projection block's matmul groups so the PE stream
            # never waits on a fresh DVE result
            carry = []

            def flush_carry(n=1):
                for _ in range(min(n, len(carry))):
                    carry.pop(0)()

            def proj_units(tb):
                """Projection work for t-block tb as PE-unit emitters; the
                x/weight DMAs are issued immediately at creation."""
                tsl = slice(tb * TB, (tb + 1) * TB)
                # ---- x^T stream: 4 quarter-tiles of the contraction ---
                xh = []
                for qtr in range(4):
                    xb = xs.tile(
                        [P, NCT // 4, TB], MMDT, tag="xb", name=f"xb{qtr}"
                    )
                    if tb == 0 and qtr == 0:
                        # fine-grained need order, so the first K matmuls
                        # start as soon as the leading chunks land
                        nc.sync.dma_start(xb[:, 0:1], xR[:, tb, qtr, 0:TB])
                        nc.sync.dma_start(wk_sb[:, 2:8], wkR[:, 2:8])
                        nc.sync.dma_start(xb[:, 1:2], xR[:, tb, qtr, TB : 2 * TB])
                        nc.sync.dma_start(wk_sb[:, 8:16], wkR[:, 8:16])
                        nc.sync.dma_start(
                            xb[:, 2:4], xR[:, tb, qtr, 2 * TB : 4 * TB]
                        )
                        # V weights next: the V units run right after K
                        nc.sync.dma_start(wv_sb[:], wvR[:])
                    else:
                        nc.sync.dma_start(xb[:], xR[:, tb, qtr, :])
                    xh.append(xb)

                def xc(c):
                    return xh[c // (NCT // 4)][:, c % (NCT // 4), :]

                if tb == 0:
                    # small later-needed loads, on the ACT queue
                    nc.scalar.dma_start(tri_sb[:], tri[:])
                    nc.scalar.dma_start(id_sb[:], ident[:])
                    nc.scalar.dma_start(v_sb[:, :, :, D], vones[:])
                if tb == 1:
                    nc.sync.dma_start(wc_sb[:], wcR[:])

                # rope staging: 5 projection tiles (K, Q0..3) in fp16
                tmp5 = rotp.tile([P, 5, TB], F16, tag="rp_t")
                rtmp5 = rotp.tile([P, 5, TB], F16, tag="rp_r")

                # ---- K^T projection (one [128, TB] tile: 2 kv heads) --
                def rope(i, eng=None):
                    # K's rope splits its first mul onto the Pool engine so
                    # attention never waits on the DVE backlog for k_sb,
                    # without a long Pool chain delaying the causal masks
                    e = eng or nc.vector
                    dst = k_sb[:, tsl] if i == 0 else q_sb[:, i - 1, tsl]
                    e.tensor_mul(dst, tmp5[:, i, :], cos_sb[:, tsl])
                    nc.vector.tensor_mul(
                        rtmp5[:, i, :], rtmp5[:, i, :], sin_sb[:, tsl]
                    )
                    nc.vector.tensor_add(dst, dst, rtmp5[:, i, :])

                def rotate(lo, hi):
                    for olo, ilo in ((0, 32), (32, 0), (64, 96), (96, 64)):
                        nc.sync.dma_start(
                            rtmp5[olo : olo + 32, lo:hi, :],
                            tmp5[ilo : ilo + 32, lo:hi, :],
                        )

                def k_unit():
                    # K^T projection (one [128, TB] tile: 2 kv heads)
                    if tb == 0:
                        nc.sync.dma_start(cos_sb[:], cosT[:])
                        nc.sync.dma_start(sin_sb[:], sinT[:])
                    pk = psMM.tile([P, TB], F32, tag="mm512", name="pk")
                    for c in range(NCT):
                        nc.tensor.matmul(
                            pk[:], wk_sb[:, c, :], xc(c),
                            start=(c == 0), stop=(c == NCT - 1),
                        )
                    nc.vector.tensor_copy(tmp5[:, 0, :], pk[:])
                    # K rope first so attention can start before Q finishes
                    rotate(0, 1)
                    rope(0, eng=nc.gpsimd)

                def v_unit(s):
                    # V: direct [t, d] projection (lhsT = x chunk)
                    if tb == 0:
                        nc.sync.dma_start(wq_sb[:, s], wqR[:, s])
                    kt = tb * (TB // P) + s
                    pvd = psMM.tile([P, 2 * D], F32, tag="mm512", name="pvd")
                    for c in range(NCT):
                        nc.tensor.matmul(
                            pvd[:], xc(c)[:, s * P : (s + 1) * P],
                            wv_sb[:, c, :],
                            start=(c == 0), stop=(c == NCT - 1),
                        )
                    nc.vector.tensor_copy(
                        v_sb[:, kt, :, 0:D],
                        pvd[:].rearrange("p (h d) -> p h d", h=2),
                    )

                def q_unit(m):
                    pq = psMM.tile([P, TB], F32, tag="mm512", name="pq")
                    for c in range(NCT):
                        nc.tensor.matmul(
                            pq[:], wq_sb[:, m, c, :], xc(c),
                            start=(c == 0), stop=(c == NCT - 1),
                        )
                    nc.vector.tensor_copy(tmp5[:, 1 + m, :], pq[:])
                    if m == 1:
                        rotate(1, 3)
                        rope(1)
                        rope(2)
                    if m == 3:
                        rotate(3, 5)
                        rope(3)
                        rope(4)

                return (
                    [k_unit]
                    + [lambda s=s: v_unit(s) for s in range(4)]
                    + [lambda m=m: q_unit(m) for m in range(4)]
                )

            def outproj_units(jq):
                # ---- output projection for rows jq*TB..(jq+1)*TB, as 8
                # independent emitters (one [128, 1024] slab each) that the
                # next attention block interleaves into its PE stream ------
                obs = {}
                split = jq == NTB - 1

                def po_half(s, half, cbh):
                    # one [128, 512] column block: finer filler units keep
                    # the PE stream alternating between long matmuls and
                    # the weight-load-bound attention PV matmuls
                    t = jq * 4 + s
                    tsl = slice(t * P, (t + 1) * P)
                    split_store = split and s == 3
                    if cbh == 0:
                        obs[(s, half)] = ostage.tile(
                            [P, 2, 512], BF16, tag="ob", name="ob"
                        )
                    ob = obs[(s, half)]
                    cb = half * 2 + cbh
                    csl = slice(cb * 512, (cb + 1) * 512)
                    po = psMM.tile([P, 512], F32, tag="mm512", name="po")
                    for jj in range(4):
                        nc.tensor.matmul(
                            po[:],
                            y_sb[:, jj, tsl],
                            wc_sb[:, jj, csl],
                            start=(jj == 0),
                            stop=(jj == 3),
                        )
                        if jj < 3:
                            # generator midpoint: lets attention PV matmuls
                            # interleave between these long streams so the
                            # PE weight-load pull-ahead hides the PV
                            # LDWEIGHTS (HW effect; PSUM-safe because only
                            # psPV-bank matmuls run while this group is
                            # open, and the driver closes the generator
                            # before any other psMM allocation)
                            yield
                    nc.vector.tensor_copy(ob[:, cbh, :], po[:])
                    if split_store:
                        # tail: store each half-slab as soon as its copy
                        # lands, via the low-latency HWDGE queue
                        nc.sync.dma_start(out[tsl, csl], ob[:, cbh, :])
                    elif cbh == 1:
                        # all stores on the SP HWDGE queue: SWDGE stores
                        # would serialize with the causal masks on Pool
                        nc.sync.dma_start(
                            out[tsl, half * 1024 : (half + 1) * 1024], ob[:]
                        )

                return [
                    (lambda s=s, half=half, cbh=cbh: po_half(s, half, cbh))
                    for s in range(4)
                    for half in range(2)
                    for cbh in range(2)
                ]

            def attn_block(jq, fillers, tail_units=None):
                qb = jq * QB
                nkt = 4 * jq + 4
                niter = 4 * nkt
                nfill = len(fillers)
                it = 0
                delayed = []  # (emit_at_iteration, kind, pr, s, callable)
                gens = []  # mid-flight filler generators (split po units)

                def run_unit(u):
                    r = u()
                    if hasattr(r, "__next__"):
                        for _ in r:
                            pass

                def gen_step():
                    # advance one split filler by one 2-matmul burst
                    if gens:
                        try:
                            next(gens[0])
                        except StopIteration:
                            gens.pop(0)

                def gen_finish():
                    # close any open filler group before a psMM allocation
                    while gens:
                        try:
                            next(gens[0])
                        except StopIteration:
                            gens.pop(0)

                def flush_delayed(force=False):
                    while delayed and (force or delayed[0][0] <= it):
                        _, kind, dpr, ds, fn = delayed.pop(0)
                        fn()
                        if kind == "tr" and tail_units is not None and dpr == 3:
                            # final block: rows for subtile ds are complete
                            # across all head pairs -> emit its out-proj
                            for u in tail_units.pop(ds, []):
                                run_unit(u)

                for pr in range(4):  # head-pair tiles (local heads pr, pr+4)
                    # PV psum tiles are allocated AFTER the previous pr's
                    # drained readouts (see below) so the pool registers
                    # those reads as write-after-read predecessors
                    pvbox = {}

                    def pvt(s, pvbox=pvbox):
                        return pvbox["AB"[s >= 2]][:, s % 2]

                    def st_unit(kt, pr=pr):
                        # scores S^T for k-tile kt + exp (bf16 PSUM)
                        j = kt - 4 * jq
                        w = QB - P * j if j >= 0 else QB
                        qoff = qb + P * j if j >= 0 else qb
                        ksl = slice(kt * P, (kt + 1) * P)
                        st = psST.tile([P, 2, QB], F32, tag="st")
                        for hh in range(2):
                            hsl = slice(hh * D, (hh + 1) * D)
                            nc.tensor.matmul(
                                st[:, hh, 0:w],
                                k_sb[hsl, ksl],
                                q_sb[hsl, pr, qoff : qoff + w],
                                start=True,
                                stop=True,
                            )
                        ptile = ptp.tile([P, 2, QB], MMDT, tag="pt")
                        nc.scalar.activation(
                            ptile[:, :, 0:w],
                            st[:, :, 0:w],
                            mybir.ActivationFunctionType.Exp,
                            scale=EXP_SCALE,
                        )
                        if j >= 0:
                            nc.gpsimd.tensor_mul(
                                ptile[:, :, 0:P],
                                ptile[:, :, 0:P],
                                tri_sb[:, None, :].to_broadcast((P, 2, P)),
                            )
                        return ptile

                    def readout(s, pvt=pvt):
                        # 1/denominator, both heads in one op
                        rec = small.tile([P, 2], F32, tag="rec")
                        nc.vector.reciprocal_approx_fast(
                            rec[:], pvt(s)[:, :, D : D + 1]
                        )
                        yqd = yqdp.tile([P, 2 * D], MMDT, tag="yqd")
                        for hh in range(2):
                            nc.vector.tensor_scalar_mul(
                                yqd[:, hh * D : (hh + 1) * D],
                                pvt(s)[:, hh, 0:D],
                                rec[:, hh : hh + 1],
                            )
                        return yqd

                    def transpose_out(yqd, s, pr=pr):
                        # yqd was produced by readout(s); PE-transpose it
                        # into the y^T layout the output projection wants
                        ytr = psMM.tile([P, P], MMDT, tag="mm512", name="ytr")
                        nc.tensor.transpose(ytr[:], yqd[:], id_sb[:])
                        # exp is done by the last pr of the last block, so
                        # ACT is free there while DVE still has a backlog
                        eng = (
                            nc.scalar.copy
                            if (tail_units is not None and pr == 3)
                            else nc.vector.tensor_copy
                        )
                        eng(
                            y_sb[:, pr, qb + s * P : qb + (s + 1) * P], ytr[:]
                        )

                    def pv_mm(ptile, kt, s, j, pvt=pvt):
                        off = (s - max(j, 0)) * P
                        for hh in range(2):
                            # PSUM 'start' zeroes the whole bank's
                            # has_written bits, so exactly one start/stop
                            # per bank (pair of subtiles): intermediate
                            # writes overwrite-or-accumulate via the
                            # per-element has_written bits
                            nc.tensor.matmul(
                                pvt(s)[:, hh, 0 : D + 1],
                                ptile[:, hh, off : off + P],
                                v_sb[:, kt, hh, :],
                                start=(kt == 0 and s % 2 == 0 and hh == 0),
                                stop=(j == s and s % 2 == 1 and hh == 1),
                            )

                    def readout_pair(j, pr=pr, readout=readout, transpose_out=transpose_out):
                        # the bank holding subtiles (j-1, j) just closed
                        for s in (j - 1, j):
                            yqd = readout(s)
                            delayed.append(
                                (
                                    it + 4,
                                    "tr",
                                    pr,
                                    s,
                                    lambda y=yqd, s=s, f=transpose_out: f(y, s),
                                )
                            )

                    def diag_pv(p, kt, jj, pv_mm=pv_mm, readout_pair=readout_pair):
                        pv_mm(p, kt, jj, jj)
                        if jj in (1, 3):
                            readout_pair(jj)

                    def drain_pv(dpr):
                        # dpr's PSUM banks must close before this pr
                        # reopens them: run dpr's deferred diagonal-PV
                        snap = delayed[:]
                        del delayed[:]
                        kept = []
                        for e in snap:
                            if e[1] == "pv" and e[2] == dpr:
                                e[4]()  # may append fresh 'tr' entries
                            else:
                                kept.append(e)
                        delayed[:] = kept + delayed

                    ptile = st_unit(0)
                    if pr == 0:
                        # transposes carried from the previous block flush
                        # behind the first scores so the PE has work while
                        # their DVE inputs land
                        flush_carry(len(carry))
                    else:
                        # behind this pr's first scores: the Pool mask for
                        # the previous pr's last diagonal gets covered
                        drain_pv(pr - 1)
                    # PV psums, q on partitions: [q, subtile-pair, head,
                    # 128] padded to a full 2KB bank so matmul regions
                    # never straddle a bank boundary
                    pvbox["A"] = psPV.tile([P, 2, 2, P], F32, tag="pvA", name="pvA")
                    pvbox["B"] = psPV.tile([P, 2, 2, P], F32, tag="pvB", name="pvB")
                    prev = None  # (ptile, kt, j) whose PV runs next iter
                    for kt in range(nkt):
                        j = kt - 4 * jq  # >= 0: diagonal-crossing tile
                        nxt = st_unit(kt + 1) if kt + 1 < nkt else None
                        if j >= 0 and kt == 0:
                            # jq=0 only: the bank 'start' rides this diag
                            # matmul, so it must be emitted first
                            diag_pv(ptile, kt, j)
                        # non-diagonal PV runs one iteration late so exp
                        # always has a full S^T of slack; split filler
                        # bursts alternate with the PV pairs so the PE
                        # weight-load pull-ahead can hide PV LDWEIGHTS
                        if prev is not None:
                            pp, pkt, pj = prev
                            for s in range(
                                max(pj, 0) + (1 if pj >= 0 else 0), 4
                            ):
                                pv_mm(pp, pkt, s, pj)
                                gen_step()
                        if j >= 0 and kt > 0:
                            # the diagonal chunk additionally needs the
                            # Pool-engine mask; two iterations of slack
                            delayed.append(
                                (
                                    it + 3,
                                    "pv",
                                    pr,
                                    j,
                                    lambda p=ptile, kt=kt, jj=j, f=diag_pv: f(
                                        p, kt, jj
                                    ),
                                )
                            )
                        prev = (ptile, kt, j)
                        ptile = nxt
                        it += 1
                        gen_finish()  # close open groups before psMM allocs
                        flush_delayed()
                        # consume fillers proportionally across the block
                        while fillers and (nfill - len(fillers)) * niter < it * nfill:
                            g = fillers.pop(0)()
                            if hasattr(g, "__next__"):
                                try:
                                    next(g)  # first burst now, rest next iter
                                    gens.append(g)
                                except StopIteration:
                                    pass

                    gen_finish()
                    if prev is not None:
                        pp, pkt, pj = prev
                        for s in range(max(pj, 0) + (1 if pj >= 0 else 0), 4):
                            pv_mm(pp, pkt, s, pj)
                    if pr == 3:
                        drain_pv(pr)

                if tail_units is not None:
                    flush_delayed(force=True)
                else:
                    # run leftover diagonal-PV work now (it can append
                    # fresh readout/transpose entries); only transposes
                    # are carried into the next block's PE stream
                    while delayed:
                        _, kind, dpr, ds, fn = delayed.pop(0)
                        if kind == "pv":
                            fn()
                        else:
                            carry.append(fn)
                for fn in fillers:
                    run_unit(fn)
                del fillers[:]

            for u in proj_units(0):
                u()
            fillers = proj_units(1)
            for tb in range(NTB):
                tail = None
                if tb == NTB - 1:
                    units = outproj_units(tb)
                    tail = {s: units[4 * s : 4 * s + 4] for s in range(4)}
                attn_block(tb, fillers, tail)
                # Filler rebalance across the remaining attention blocks:
                # attn(1) gets proj(2)+out(0); attn(2) gets proj(3) only
                # (it is oversupplied anyway); attn(3) - whose exp overhang
                # is the largest - gets BOTH out(1) and out(2).
                a = proj_units(tb + 2) if tb + 2 < NTB else []
                if tb == 0:
                    b = outproj_units(0)
                elif tb == 1:
                    held = outproj_units(1)
                    b = held[:4]  # 6 units now; the rest feeds attn(3)
                elif tb == 2:
                    b = held[4:] + outproj_units(2)
                else:
                    b = []
                fillers = []
                while a or b:
                    if a:
                        fillers.append(a.pop(0))
                    if b:
                        fillers.append(b.pop(0))
            flush_carry(len(carry))

    nc.finalize()
    return nc


def _rope_tables(position_ids):
    t = position_ids.reshape(-1).astype(np.float64)  # [T]
    inv_freq = 1.0 / ROPE_THETA ** (np.arange(0, D, 2, dtype=np.float64) / D)
    freqs = np.outer(t, inv_freq)  # [T, D/2]
    cos = np.repeat(np.cos(freqs), 2, axis=1)  # [T, D] interleaved
    sin = np.repeat(np.sin(freqs), 2, axis=1)
    sign = np.where(np.arange(D) < D // 2, -1.0, 1.0)
    cosT = np.tile(cos.T, (2, 1)).astype(np.float16)            # [128, T]
    sinT = np.tile((sin * sign).T, (2, 1)).astype(np.float16)   # [128, T]
    return np.ascontiguousarray(cosT), np.ascontiguousarray(sinT)


def _head_perm(g):
    # row indices into Wq (and columns of Wc) for core head-group g
    rows = []
    for lh in LOCAL_HEADS:
        h = g * QH + lh
        rows.extend(range(h * D, (h + 1) * D))
    return np.asarray(rows)


def _part_major(wT, width):
    """[C, width] -> [128, NCT*width] with per-partition contiguous lines
    (line (c, m) = wT[c*128+p, m])."""
    return np.ascontiguousarray(
        wT.reshape(NCT, P, width).transpose(1, 0, 2).reshape(P, NCT * width)
    )


def make_in_maps(x, Wq, Wk, Wv, Wc, position_ids):
    x = np.asarray(x, dtype=np.float32)
    Wq = np.asarray(Wq, dtype=np.float32)
    Wk = np.asarray(Wk, dtype=np.float32)
    Wv = np.asarray(Wv, dtype=np.float32)
    Wc = np.asarray(Wc, dtype=np.float32)
    cosT, sinT = _rope_tables(np.asarray(position_ids))
    tri = np.triu(np.ones((P, P), dtype=np.float32))  # allow q >= k
    in_maps = []
    xr_cache = {}
    for core in range(8):
        b, g = divmod(core, 4)
        perm = _head_perm(g)
        kv = slice(2 * g * D, (2 * g + 2) * D)
        if b not in xr_cache:
            # xR[p, tb, qtr, o4*512+t] = x[b][tb*512+t, (qtr*4+o4)*128+p]
            xT = np.ascontiguousarray(x[b].T).astype(NPDT)  # [C, T]
            xr_cache[b] = np.ascontiguousarray(
                xT.reshape(4, 4, P, NTB, TB).transpose(2, 3, 0, 1, 4)
                .reshape(P, NTB, 4, 4 * TB)
            )
        wqT = np.ascontiguousarray(Wq[perm].T).astype(NPDT)  # [C, 512]
        in_maps.append(
            {
                "xR": xr_cache[b],
                # wqR[p, m, c, mm] = WqT[c*128+p, m*128+mm]
                "wqR": np.ascontiguousarray(
                    wqT.reshape(NCT, P, 4, P).transpose(1, 2, 0, 3)
                ),
                "wkR": _part_major(
                    np.ascontiguousarray(Wk[kv].T).astype(NPDT), 2 * D
                ),
                "wvR": _part_major(
                    np.ascontiguousarray(Wv[kv].T).astype(NPDT), 2 * D
                ),
                "wcR": np.ascontiguousarray(
                    Wc[:, perm].T.reshape(4, P, C).transpose(1, 0, 2)
                    .reshape(P, 4 * C)
                ).astype(NPDT),
                "cosT": cosT,
                "sinT": sinT,
                "tri": tri.astype(NPDT),
                "ident": np.eye(P, dtype=np.float32).astype(NPDT),
                "vones": np.ones((P, KT, 2), dtype=NPDT),
            }
        )
    return in_maps


_NC = None


def get_nc():
    global _NC
    if _NC is None:
        _NC = build_bass()
    return _NC


def run_cores(in_maps, core_ids, **kw):
    return run_bass_kernel_spmd(get_nc(), in_maps, core_ids=core_ids, **kw)


def kernel(x, Wq, Wk, Wv, Wc, position_ids, _trace=False, _res_out=None):
    in_maps = make_in_maps(x, Wq, Wk, Wv, Wc, position_ids)
    res = run_cores(in_maps, list(range(8)), trace=_trace)
    if _res_out is not None:
        _res_out.append(res)
    outs = [
        np.asarray(res.results[i]["out"], dtype=np.float32) for i in range(8)
    ]
    y = np.stack(
        [
            outs[0] + outs[1] + outs[2] + outs[3],
            outs[4] + outs[5] + outs[6] + outs[7],
        ]
    )
    return y.astype(np.float32)

